# revision 12
# baseline (speedup 1.0000x reference)
"""Bilinear pooling kernel for 8 Trainium2 NeuronCores (Bass/Tile).

Math (matches the jax reference):
  x = concat([x1, x2, x3], channel) -> (B=64, M=147, L=3136)
  phi_b = x_b @ x_b.T                              (147, 147), symmetric
  phi = sign(phi) * sqrt(|phi| + EPS)              (signed sqrt)
  phi = phi / sqrt(sum(phi^2 + EPS) + 1.0)         (per-batch normalize)
  h = phi_vec @ fc0_w.T + fc0_b                    (64, 1024)
  y = h @ fc1_w.T + fc1_b                          (64, 64)
  logits = y @ fc2_w.T + fc2_b                     (64, 4)
  merged = softmax(concat([logits, x11, x21, x31]))
  x_merge = merged @ cls_w.T + cls_b               (64, 4)
  returns (logits, x_merge)

Distribution:
  phase 1: batch-parallel bilinear+signed-sqrt+normalize (8 batches/core)
  phase 2: AllGather normalized phi (padded to 21632 cols; col 21609 == 1.0
           so fc0_b rides as an extra W^T row -- no separate bias add)
  phase 3: PE-transpose phi to (i, b) layout; fc0 with output-column-sharded
           W^T (128 of 1024 outputs per core); fc1 partial contraction
  phase 4: AllReduce the (64, 64) y^T partials
  phase 5: replicated tail (fc2, softmax, cls); outputs read from core 0
"""

import sys

sys.path.insert(0, "/opt/trn_rl_repo")

import numpy as np

import concourse.bass as bass
import concourse.tile as tile
from concourse import masks, mybir
from concourse.bass_utils import run_bass_kernel_spmd
import bass_rust
from bass_rust import ScopedClock

# ---------------------------------------------------------------------------
# Workaround: this toolchain's walrus accepts only ONE semaphore wait on a
# CTRL (Drain) instruction, but Tile's kernel-tail drain aggregates every
# outstanding semaphore onto one drain.  Split the excess waits onto
# trailing SP nops (same engine => executed in order before the end
# barrier, so semantics are unchanged).
# ---------------------------------------------------------------------------
_TAIL_MAX_WAITS = 1


def _split_drain_and_barrier(self, tick_clock, wait_clock):
    nc = self.nc
    drain_inst = nc.sync.drain()
    wait_clock.add_sem_waits(
        drain_inst.ins, ScopedClock({None: tick_clock.global_clock})
    )
    info = drain_inst.ins.sync_info
    if info is not None and len(info.on_wait) > _TAIL_MAX_WAITS:
        waits = list(info.on_wait)
        drain_inst.ins.sync_info = bass_rust.SyncInfo(
            on_wait=waits[:_TAIL_MAX_WAITS], on_update=list(info.on_update)
        )
        rest = waits[_TAIL_MAX_WAITS:]
        while rest:
            chunk, rest = rest[:_TAIL_MAX_WAITS], rest[_TAIL_MAX_WAITS:]
            nop_inst = nc.sync.nop(nofuse=True, hint="tail_drain_split")
            nop_inst.ins.sync_info = bass_rust.SyncInfo(on_wait=chunk, on_update=[])
    nc.all_engine_barrier()
    assert self.sems is not None
    popped = nc._tile_sem_poison_stack.pop()
    assert popped is self._sem_poison
    nc.clear_and_free_semaphores(list(self.sems.allocated().values()))
    nc.all_engine_barrier()


tile.TileContext._drain_and_barrier = _split_drain_and_barrier


_RealTCW = tile.TileClockWait
_ws_counter = [0]


def _split_excess_waits(obb):
    """Move excess semaphore waits (>1 per instruction) onto same-engine
    nops placed immediately before the instruction."""
    for bb, insts in list(obb.items()):
        new_list = []
        for inst in insts:
            info = inst.sync_info
            if info is not None and len(info.on_wait) > _TAIL_MAX_WAITS:
                waits = list(info.on_wait)
                excess = waits[:-_TAIL_MAX_WAITS]
                keep = waits[-_TAIL_MAX_WAITS:]
                for i in range(0, len(excess), _TAIL_MAX_WAITS):
                    _ws_counter[0] += 1
                    nop = mybir.InstNoOp(
                        name=f"WS-{_ws_counter[0]}",
                        sync_info=bass_rust.SyncInfo(
                            on_wait=excess[i : i + _TAIL_MAX_WAITS],
                            on_update=[],
                        ),
                        bass_nofuse=True,
                        engine=inst.engine,
                    )
                    new_list.append(nop)
                inst.sync_info = bass_rust.SyncInfo(
                    on_wait=keep, on_update=list(info.on_update)
                )
            new_list.append(inst)
        obb[bb] = new_list


class _TCWWrapper:
    """Delegating wrapper around the Rust TileClockWait that splits excess
    waits after assignment (this walrus accepts only one wait/instruction)."""

    def __init__(self, *args, **kwargs):
        self._inner = _RealTCW(*args, **kwargs)
        self._obb = args[1] if len(args) > 1 else kwargs["ordered_instructions_by_block"]

    def __getattr__(self, name):
        return getattr(self._inner, name)

    def assign_waits(self, bb_name):
        self._inner.assign_waits(bb_name)
        _split_excess_waits(self._obb)


tile.TileClockWait = _TCWWrapper

# ---------------------------------------------------------------------------
# Problem constants (hardcoded per the spec)
# ---------------------------------------------------------------------------
N_CORES = 8
CORE_IDS = list(range(N_CORES))
B = 64
B_LOC = B // N_CORES  # 8 batches per core
C = 49
L = 3136  # 56*56
M = 147  # 3*49 channels
MM = M * M  # 21609
NI_CHUNKS = 169  # ceil((MM+1)/128): includes the bias column at i=21609
MM_PAD = NI_CHUNKS * 128  # 21632
O0 = 1024  # fc0 out features
O0_LOC = O0 // N_CORES  # 128 per core
HID = 64  # fc1 out features
CLS = 4
EPS = 1e-8
# normalizer constant: sum(phi_ss^2 + EPS) + 1.0 == sum|phi| + 2*MM*EPS + 1.0
NORM_C = float(2 * MM * EPS + 1.0)

LFULL = 24  # full 128-row l-chunks
LTAIL = 64  # tail chunk rows (3136 = 24*128 + 64)

F32 = mybir.dt.float32


def _build_nc():
    nc = bass.Bass()

    # -- external I/O ------------------------------------------------------
    # x shards arrive host-pretransposed to (b, l, m) so device loads are
    # contiguous along the DMA's innermost dim
    x1_d = nc.dram_tensor("x1", [B_LOC, L, C], F32, kind="ExternalInput")
    x2_d = nc.dram_tensor("x2", [B_LOC, L, C], F32, kind="ExternalInput")
    x3_d = nc.dram_tensor("x3", [B_LOC, L, C], F32, kind="ExternalInput")
    x11_d = nc.dram_tensor("x11", [B, CLS], F32, kind="ExternalInput")
    x21_d = nc.dram_tensor("x21", [B, CLS], F32, kind="ExternalInput")
    x31_d = nc.dram_tensor("x31", [B, CLS], F32, kind="ExternalInput")
    w0t_d = nc.dram_tensor("w0t", [MM_PAD, O0_LOC], F32, kind="ExternalInput")
    w1t_d = nc.dram_tensor("w1t", [O0_LOC, HID], F32, kind="ExternalInput")
    fc1b_d = nc.dram_tensor("fc1b", [HID, 1], F32, kind="ExternalInput")
    w2t_d = nc.dram_tensor("w2t", [HID + 1, CLS], F32, kind="ExternalInput")
    wct_d = nc.dram_tensor("wct", [4 * CLS + 1, CLS], F32, kind="ExternalInput")
    logits_d = nc.dram_tensor("logits", [B, CLS], F32, kind="ExternalOutput")
    xmerge_d = nc.dram_tensor("x_merge", [B, CLS], F32, kind="ExternalOutput")

    xs_d = [x1_d, x2_d, x3_d]

    with tile.TileContext(nc) as tc:
        with tc.tile_pool(name="dram", bufs=1, space="DRAM") as dram, tc.tile_pool(
            name="const", bufs=1
        ) as const:
            # -- collective buffers -------------------------------------
            phi_cont = dram.tile([B_LOC, MM_PAD], F32)
            phi_all = dram.tile([B, MM_PAD], F32, addr_space="Shared")
            yt_part = dram.tile([HID, B], F32)
            yt_full = dram.tile([HID, B], F32, addr_space="Shared")

            # -- constants ----------------------------------------------
            ident = const.tile([128, 128], F32)
            masks.make_identity(nc, ident[:])
            ones_col = const.tile([128, 128], F32)
            nc.gpsimd.memset(ones_col[:], 1.0)
            # bias/pad tail for phi rows: [1.0, 0 x 22]
            tail_pat = const.tile([1, MM_PAD - MM], F32)
            nc.gpsimd.memset(tail_pat[:], 0.0)
            nc.gpsimd.memset(tail_pat[:, 0:1], 1.0)
            eps_col = const.tile([128, 1], F32)
            nc.gpsimd.memset(eps_col[:], EPS)
            normc_col = const.tile([128, 1], F32)
            nc.gpsimd.memset(normc_col[:], NORM_C)

            # ===========================================================
            # phase 1: bilinear + signed sqrt + normalize, per batch
            # ===========================================================
            with tc.tile_pool(name="xt", bufs=2) as xt_pool, tc.tile_pool(
                name="p1sb", bufs=2
            ) as sb, tc.tile_pool(name="p1ps", bufs=2, space="PSUM") as ps:
                for b in range(B_LOC):
                    # xt[p, lc, m] = xT[b, 128*lc + p, m]
                    xt = xt_pool.tile([128, LFULL, M], F32, tag="xt")
                    xtt = xt_pool.tile([LTAIL, M], F32, tag="xtt")
                    for j in range(3):
                        src = xs_d[j][b]
                        nc.sync.dma_start(
                            xt[:, :, 49 * j : 49 * j + 49],
                            src[0 : 128 * LFULL].rearrange(
                                "(lc p) m -> p lc m", p=128
                            ),
                        )
                        nc.sync.dma_start(
                            xtt[:, 49 * j : 49 * j + 49],
                            src[128 * LFULL : L],
                        )

                    # phi blocks: A = rows 0:128 x cols 0:147,
                    #             Bk = rows 128:147 x cols 128:147
                    pA = ps.tile([128, M], F32, tag="pA")
                    pB = ps.tile([M - 128, M - 128], F32, tag="pB")
                    for lc in range(LFULL + 1):
                        if lc < LFULL:
                            lhs_a = xt[:, lc, 0:128]
                            rhs_a = xt[:, lc, :]
                            lhs_b = xt[:, lc, 128:M]
                        else:
                            lhs_a = xtt[:, 0:128]
                            rhs_a = xtt[:, :]
                            lhs_b = xtt[:, 128:M]
                        nc.tensor.matmul(
                            pA[:], lhs_a, rhs_a,
                            start=(lc == 0), stop=(lc == LFULL),
                        )
                        nc.tensor.matmul(
                            pB[:], lhs_b, lhs_b,
                            start=(lc == 0), stop=(lc == LFULL),
                        )

                    # signed sqrt pieces
                    sgnA = sb.tile([128, M], F32, tag="sgnA")
                    absA = sb.tile([128, M], F32, tag="absA")
                    sgnB = sb.tile([M - 128, M - 128], F32, tag="sgnB")
                    absB = sb.tile([M - 128, M - 128], F32, tag="absB")
                    nc.scalar.activation(sgnA[:], pA[:], mybir.ActivationFunctionType.Sign)
                    nc.scalar.activation(absA[:], pA[:], mybir.ActivationFunctionType.Abs)
                    nc.scalar.activation(sgnB[:], pB[:], mybir.ActivationFunctionType.Sign)
                    nc.scalar.activation(absB[:], pB[:], mybir.ActivationFunctionType.Abs)

                    # row sums of |phi| (for the normalizer):
                    # total = sum(A) + sum(A[:, 128:147]) + sum(B)
                    rsA = sb.tile([128, 1], F32, tag="rsA")
                    rsAs = sb.tile([128, 1], F32, tag="rsAs")
                    rsB = sb.tile([M - 128, 1], F32, tag="rsB")
                    nc.vector.reduce_sum(rsA[:], absA[:], axis=mybir.AxisListType.X)
                    nc.vector.reduce_sum(
                        rsAs[:], absA[:, 128:M], axis=mybir.AxisListType.X
                    )
                    nc.vector.reduce_sum(rsB[:], absB[:], axis=mybir.AxisListType.X)

                    # ss = sign * sqrt(|phi| + EPS)
                    sqA = sb.tile([128, M], F32, tag="sqA")
                    sqB = sb.tile([M - 128, M - 128], F32, tag="sqB")
                    nc.scalar.activation(
                        sqA[:], absA[:], mybir.ActivationFunctionType.Sqrt, bias=eps_col[:]
                    )
                    nc.scalar.activation(
                        sqB[:], absB[:], mybir.ActivationFunctionType.Sqrt, bias=eps_col[0 : M - 128]
                    )
                    ssA = sb.tile([128, M], F32, tag="ssA")
                    ssB = sb.tile([M - 128, M - 128], F32, tag="ssB")
                    nc.vector.tensor_mul(ssA[:], sqA[:], sgnA[:])
                    nc.vector.tensor_mul(ssB[:], sqB[:], sgnB[:])

                    # cross-partition sum + broadcast in one accumulation
                    # group: bc[m] = sum_k ones[k, m] * rs[k]
                    bc = ps.tile([128, 1], F32, tag="bc")
                    nc.tensor.matmul(bc[:], ones_col[:, :], rsA[:], start=True, stop=False)
                    nc.tensor.matmul(bc[:], ones_col[:, :], rsAs[:], start=False, stop=False)
                    nc.tensor.matmul(
                        bc[:], ones_col[0 : M - 128, :], rsB[:], start=False, stop=True
                    )

                    # scale = 1 / sqrt(total + NORM_C)
                    inv = sb.tile([128, 1], F32, tag="inv")
                    nc.scalar.activation(
                        inv[:], bc[:], mybir.ActivationFunctionType.Sqrt, bias=normc_col[:]
                    )
                    scl = sb.tile([128, 1], F32, tag="scl")
                    nc.vector.reciprocal(scl[:], inv[:])

                    nA = sb.tile([128, M], F32, tag="nA")
                    nB = sb.tile([M - 128, M - 128], F32, tag="nB")
                    nc.vector.tensor_scalar_mul(nA[:], ssA[:], scl[:])
                    nc.vector.tensor_scalar_mul(nB[:], ssB[:], scl[0 : M - 128])

                    # rows 128:147 cols 0:128 of phi == nA[:, 128:147].T
                    pT = ps.tile([M - 128, 128], F32, tag="pT")
                    nc.tensor.transpose(pT[:], nA[:, 128:M], ident[:])
                    nT = sb.tile([M - 128, 128], F32, tag="nT")
                    nc.scalar.copy(nT[:], pT[:])

                    # write phi row (flattened, m-major) + bias/pad tail
                    row = phi_cont[b]
                    nc.sync.dma_start(
                        row[0 : 128 * M].rearrange("(m n) -> m n", n=M), nA[:]
                    )
                    nc.sync.dma_start(
                        row[128 * M : MM].rearrange("(m n) -> m n", n=M)[:, 0:128],
                        nT[:],
                    )
                    nc.sync.dma_start(
                        row[128 * M : MM].rearrange("(m n) -> m n", n=M)[:, 128:M],
                        nB[:],
                    )
                    nc.sync.dma_start(row[MM:MM_PAD], tail_pat[0, :])

            # ===========================================================
            # phase 2: AllGather phi
            # ===========================================================
            nc.gpsimd.collective_compute(
                "AllGather",
                mybir.AluOpType.bypass,
                replica_groups=[CORE_IDS],
                ins=[phi_cont.opt()],
                outs=[phi_all.opt()],
            )

            # ===========================================================
            # phase 3: transpose phi, fc0 (o-sharded), fc1 partial
            # ===========================================================
            with tc.tile_pool(name="p3sb", bufs=1) as sb3, tc.tile_pool(
                name="wpool", bufs=3
            ) as wpool, tc.tile_pool(name="p3ps", bufs=4, space="PSUM") as ps3, tc.tile_pool(
                name="p3ph", bufs=1, space="PSUM"
            ) as psh:
                phi_sb = sb3.tile([B, MM_PAD], F32)
                nc.sync.dma_start(phi_sb[:], phi_all[:])
                phiT = sb3.tile([128, NI_CHUNKS * B], F32)
                for k in range(NI_CHUNKS):
                    ptk = ps3.tile([128, B], F32, tag="ptk")
                    nc.tensor.transpose(
                        ptk[:], phi_sb[:, 128 * k : 128 * (k + 1)], ident[0:B, 0:B]
                    )
                    nc.vector.tensor_copy(phiT[:, B * k : B * (k + 1)], ptk[:])

                # fc0: h^T (128 o x 64 b), accumulate over 169 i-chunks
                ph = psh.tile([O0_LOC, B], F32)
                W_DMA = 8  # i-chunks per weight DMA
                n_wdma = (NI_CHUNKS + W_DMA - 1) // W_DMA  # 22 (last has 1)
                for wd in range(n_wdma):
                    k0 = wd * W_DMA
                    kn = min(W_DMA, NI_CHUNKS - k0)
                    wt = wpool.tile([128, W_DMA, O0_LOC], F32, tag="wt")
                    nc.sync.dma_start(
                        wt[:, 0:kn, :],
                        w0t_d[128 * k0 : 128 * (k0 + kn)].rearrange(
                            "(kc p) o -> p kc o", p=128
                        ),
                    )
                    for kc in range(kn):
                        k = k0 + kc
                        nc.tensor.matmul(
                            ph[:],
                            wt[:, kc, :],
                            phiT[:, B * k : B * (k + 1)],
                            start=(k == 0),
                            stop=(k == NI_CHUNKS - 1),
                        )

                h_sb = sb3.tile([O0_LOC, B], F32)
                nc.scalar.copy(h_sb[:], ph[:])

                # fc1 partial: y^T = w1t_shard.T @ h^T_shard
                w1_sb = sb3.tile([O0_LOC, HID], F32)
                nc.sync.dma_start(w1_sb[:], w1t_d[:])
                py = ps3.tile([HID, B], F32, tag="py", bufs=1)
                nc.tensor.matmul(py[:], w1_sb[:], h_sb[:], start=True, stop=True)
                yt_sb = sb3.tile([HID, B], F32)
                nc.vector.tensor_copy(yt_sb[:], py[:])
                nc.sync.dma_start(yt_part[:], yt_sb[:])

            # ===========================================================
            # phase 4: AllReduce y^T partials
            # ===========================================================
            nc.gpsimd.collective_compute(
                "AllReduce",
                mybir.AluOpType.add,
                replica_groups=[CORE_IDS],
                ins=[yt_part.opt()],
                outs=[yt_full.opt()],
            )

            # ===========================================================
            # phase 5: replicated tail
            # ===========================================================
            with tc.tile_pool(name="p5sb", bufs=1) as sb5, tc.tile_pool(
                name="p5ps", bufs=1, space="PSUM"
            ) as ps5:
                # y^T + fc1_b, augmented with a ones row for the fc2 bias
                yt_aug = sb5.tile([HID + 1, B], F32)
                ytr = sb5.tile([HID, B], F32)
                nc.sync.dma_start(ytr[:], yt_full[:])
                fc1b_sb = sb5.tile([HID, 1], F32)
                nc.sync.dma_start(fc1b_sb[:], fc1b_d[:])
                nc.scalar.activation(
                    yt_aug[0:HID, :],
                    ytr[:],
                    mybir.ActivationFunctionType.Identity,
                    bias=fc1b_sb[:],
                )
                nc.vector.tensor_copy(yt_aug[HID : HID + 1, :], ones_col[0:1, 0:B])

                w2_sb = sb5.tile([HID + 1, CLS], F32)
                nc.sync.dma_start(w2_sb[:], w2t_d[:])
                plog = ps5.tile([B, CLS], F32, tag="plog")
                nc.tensor.matmul(plog[:], yt_aug[:], w2_sb[:], start=True, stop=True)
                logit_sb = sb5.tile([B, CLS], F32)
                nc.scalar.copy(logit_sb[:], plog[:])
                nc.sync.dma_start(logits_d[:], logit_sb[:])

                # merged = [logits | x11 | x21 | x31]
                merged = sb5.tile([B, 4 * CLS], F32)
                nc.vector.tensor_copy(merged[:, 0:CLS], logit_sb[:])
                nc.sync.dma_start(merged[:, CLS : 2 * CLS], x11_d[:])
                nc.sync.dma_start(merged[:, 2 * CLS : 3 * CLS], x21_d[:])
                nc.sync.dma_start(merged[:, 3 * CLS : 4 * CLS], x31_d[:])

                # softmax over the 16 features (free dim)
                rmax = sb5.tile([B, 1], F32)
                nc.vector.reduce_max(rmax[:], merged[:], axis=mybir.AxisListType.X)
                nmax = sb5.tile([B, 1], F32)
                nc.scalar.activation(
                    nmax[:], rmax[:], mybir.ActivationFunctionType.Copy, scale=-1.0
                )
                esb = sb5.tile([B, 4 * CLS], F32)
                nc.scalar.activation(
                    esb[:], merged[:], mybir.ActivationFunctionType.Exp, bias=nmax[:]
                )
                ssum = sb5.tile([B, 1], F32)
                nc.vector.reduce_sum(ssum[:], esb[:], axis=mybir.AxisListType.X)
                rinv = sb5.tile([B, 1], F32)
                nc.vector.reciprocal(rinv[:], ssum[:])
                # softmax result with a ones column appended (becomes the
                # bias row after the transpose)
                smx = sb5.tile([B, 4 * CLS + 1], F32)
                nc.vector.tensor_scalar_mul(smx[:, 0 : 4 * CLS], esb[:], rinv[:])
                nc.vector.tensor_copy(
                    smx[:, 4 * CLS : 4 * CLS + 1], ones_col[0:B, 0:1]
                )

                # x_merge = smx @ cls_w.T + cls_b  (via transposed smx + aug)
                pmt = ps5.tile([4 * CLS + 1, B], F32, tag="pmt")
                nc.tensor.transpose(pmt[:], smx[:], ident[0:B, 0:B])
                mt_aug = sb5.tile([4 * CLS + 1, B], F32)
                nc.scalar.copy(mt_aug[:], pmt[:])

                wc_sb = sb5.tile([4 * CLS + 1, CLS], F32)
                nc.sync.dma_start(wc_sb[:], wct_d[:])
                pxm = ps5.tile([B, CLS], F32, tag="pxm")
                nc.tensor.matmul(pxm[:], mt_aug[:], wc_sb[:], start=True, stop=True)
                xm_sb = sb5.tile([B, CLS], F32)
                nc.scalar.copy(xm_sb[:], pxm[:])
                nc.sync.dma_start(xmerge_d[:], xm_sb[:])

    return nc


_NC_CACHE = None


def _get_nc():
    global _NC_CACHE
    if _NC_CACHE is None:
        _NC_CACHE = _build_nc()
    return _NC_CACHE


def _make_in_maps(inputs):
    x1 = np.ascontiguousarray(inputs["x1"], dtype=np.float32).reshape(B, C, L)
    x2 = np.ascontiguousarray(inputs["x2"], dtype=np.float32).reshape(B, C, L)
    x3 = np.ascontiguousarray(inputs["x3"], dtype=np.float32).reshape(B, C, L)
    x11 = np.ascontiguousarray(inputs["x11"], dtype=np.float32)
    x21 = np.ascontiguousarray(inputs["x21"], dtype=np.float32)
    x31 = np.ascontiguousarray(inputs["x31"], dtype=np.float32)
    fc0_w = np.asarray(inputs["fc0_w"], dtype=np.float32)
    fc0_b = np.asarray(inputs["fc0_b"], dtype=np.float32)
    fc1_w = np.asarray(inputs["fc1_w"], dtype=np.float32)
    fc1_b = np.asarray(inputs["fc1_b"], dtype=np.float32)
    fc2_w = np.asarray(inputs["fc2_w"], dtype=np.float32)
    fc2_b = np.asarray(inputs["fc2_b"], dtype=np.float32)
    cls_w = np.asarray(inputs["cls_w"], dtype=np.float32)
    cls_b = np.asarray(inputs["cls_b"], dtype=np.float32)

    w0t = np.zeros((MM_PAD, O0), dtype=np.float32)
    w0t[:MM] = fc0_w.T
    w0t[MM] = fc0_b
    w1t = np.ascontiguousarray(fc1_w.T)  # (1024, 64)
    fc1b = np.ascontiguousarray(fc1_b.reshape(HID, 1))
    w2t = np.ascontiguousarray(
        np.concatenate([fc2_w.T, fc2_b.reshape(1, CLS)], axis=0)
    )  # (65, 4)
    wct = np.ascontiguousarray(
        np.concatenate([cls_w.T, cls_b.reshape(1, CLS)], axis=0)
    )  # (17, 4)

    in_maps = []
    for c in range(N_CORES):
        sl = slice(B_LOC * c, B_LOC * (c + 1))
        ol = slice(O0_LOC * c, O0_LOC * (c + 1))
        in_maps.append(
            {
                "x1": np.ascontiguousarray(x1[sl].transpose(0, 2, 1)),
                "x2": np.ascontiguousarray(x2[sl].transpose(0, 2, 1)),
                "x3": np.ascontiguousarray(x3[sl].transpose(0, 2, 1)),
                "x11": x11,
                "x21": x21,
                "x31": x31,
                "w0t": np.ascontiguousarray(w0t[:, ol]),
                "w1t": np.ascontiguousarray(w1t[ol]),
                "fc1b": fc1b,
                "w2t": w2t,
                "wct": wct,
            }
        )
    return in_maps


def run(inputs, trace=False, **kwargs):
    nc = _get_nc()
    in_maps = _make_in_maps(inputs)
    res = run_bass_kernel_spmd(nc, in_maps, CORE_IDS, trace=trace, **kwargs)
    out = res.results[0]
    logits = np.asarray(out["logits"], dtype=np.float32)
    x_merge = np.asarray(out["x_merge"], dtype=np.float32)
    return (logits, x_merge), res


def kernel(**inputs):
    (logits, x_merge), _ = run(inputs, trace=False)
    return logits, x_merge
